# revision 56
# baseline (speedup 1.0000x reference)
"""DFlash Qwen3 cross-attention on 8 TRN2 NeuronCores.

Sharding: tensor-parallel over heads. Core c owns KV head c (KVH=8) and the
4 query heads 4c..4c+3 of its GQA group. Each core computes its heads'
QKV projections, per-head RMSNorm + RoPE, causal attention, then a FULL
[QL, HID] partial o_proj using its 4 heads' rows of w_o. The host sums the
8 partial outputs -- no device collectives at all.

All matmuls run in bf16 (fp32 PSUM accumulation); softmax in fp32 on the
scalar engine with exp fused over pairs of 512-wide q tiles ([128,1024]
ACT ops). The softmax denominator accumulates in bf16 on the vector engine
(2x DVE rate); the 128-partition noise averages out in the final
partition_all_reduce, keeping the error well inside tolerance.

RMSNorm sum-of-squares runs on the scalar engine (Square + accum_out);
RoPE multiplies are fused across the 4 heads (Q) / 4 kv chunks (K) with
the per-head norm weights folded into the host-precomputed cos/sin tables
(cA|sA|cB|sB layout).

Host-side prep: transpose ck=concat(context,query) to [HID, KV] bf16,
slice per-core weights, precompute RoPE tables and causal mask tiles.
"""

from contextlib import ExitStack

import numpy as np
from ml_dtypes import bfloat16

import concourse.bass_isa as bass_isa
import concourse.mybir as mybir
import concourse.tile as tile
from concourse import bacc
from concourse.bass_utils import run_bass_kernel_spmd

class _Bacc(bacc.Bacc):
    """Force every activation onto the one table set that covers
    exp/ln/square/copy (natural_log_exp_and_others, canonical index 6). The
    insertion pass greedily picks the first set containing each function,
    which otherwise ping-pongs table loads between the exp and ln sets.
    Canonical set indices are preserved; earlier sets are just hidden from
    the chooser."""

    def insert_act_table_loads(self):
        import bass_rust as _bass_rust
        from concourse.hw_specs import get_activation_tables

        if not any(isinstance(i, mybir.InstActivation)
                   for b in self.main_func.blocks for i in b.instructions):
            return
        tables = [(name, (set() if i < 6 else fns)) for i, (name, fns) in
                  enumerate(get_activation_tables(self.m.arch).items())]
        _bass_rust.insert_act_table_loads(self, tables)
from concourse.masks import make_identity

H = 32
KVH = 8
D = 128
HID = 4096
CTX = 4096
QL = 2048
KV = CTX + QL  # 6144
NCORES = 8
HPC = H // NCORES  # 4 q heads per core
THETA = 1000000.0
EPS = 1e-6
SCALE = float(D) ** -0.5

NHD = HID // 128  # 32 contraction chunks
NKV = KV // 128  # 48 kv chunks
NQC = QL // 128  # 16 q row chunks
NQJ = QL // 512  # 4 q column tiles for attention
MASKVAL = -1e6

F32 = mybir.dt.float32
BF16 = mybir.dt.bfloat16

_STATE = {}


def _build():
    nc = _Bacc()

    ckT = nc.declare_dram_parameter("ckT", [HID, KV], BF16, isOutput=False)
    wq = nc.declare_dram_parameter("wq", [HID, HPC * D], BF16, isOutput=False)
    wkv = nc.declare_dram_parameter("wkv", [HID, 2 * D], BF16, isOutput=False)
    wo = nc.declare_dram_parameter("wo", [HPC * D, HID], BF16, isOutput=False)
    csq = nc.declare_dram_parameter("csq", [QL, 2 * D], F32, isOutput=False)
    csk = nc.declare_dram_parameter("csk", [KV, 2 * D], F32, isOutput=False)
    msk = nc.declare_dram_parameter("msk", [128, 4 * 512], BF16, isOutput=False)
    trio = nc.declare_dram_parameter("trio", [128, 128], BF16, isOutput=False)
    out_ext = nc.declare_dram_parameter("out", [QL, HID], BF16, isOutput=True)

    with tile.TileContext(nc) as tc, ExitStack() as ctx:
        singles = ctx.enter_context(tc.tile_pool(name="singles", bufs=1))
        # streamed ckT slices for the projections
        ckq_pool = ctx.enter_context(tc.tile_pool(name="ckq", bufs=4))
        cs_pool = ctx.enter_context(tc.tile_pool(name="csp", bufs=2))
        # fp32 evacuation + norm/rope working tiles
        evac_pool = ctx.enter_context(tc.tile_pool(name="evac", bufs=2))
        tmp_pool = ctx.enter_context(tc.tile_pool(name="tmp", bufs=3))
        # attention-side pools
        p_pool = ctx.enter_context(tc.tile_pool(name="pt", bufs=5))
        sacc_pool = ctx.enter_context(tc.tile_pool(name="sacc", bufs=2))
        stg_pool = ctx.enter_context(tc.tile_pool(name="stg", bufs=2))
        ot_pool = ctx.enter_context(tc.tile_pool(name="ot", bufs=3))
        # PSUM: "acc" = 4 x 1-bank accumulators (proj groups / o_acc / po),
        # "st" = 2 x 2-bank tiles (fused S^T pairs; proj transposes reuse it)
        acc_psum = ctx.enter_context(tc.tile_pool(name="accp", bufs=2, space="PSUM"))
        st_psum = ctx.enter_context(tc.tile_pool(name="stp", bufs=2, space="PSUM"))

        # ---- resident tensors (weights needed later are loaded later) ----
        wkv_sb = singles.tile([128, NHD, 2 * D], BF16)
        nc.sync.dma_start(out=wkv_sb[:], in_=wkv[:, :].rearrange("(k p) n -> p k n", p=128))
        wq_sb = singles.tile([128, NHD, HPC * D], BF16)

        ident = singles.tile([128, 128], F32)
        make_identity(nc, ident)
        epst = singles.tile([128, 1], F32)
        nc.vector.memset(epst, EPS)
        zbias = singles.tile([128, 1], F32)
        nc.vector.memset(zbias, 0.0)

        # outputs of the projection phases (bufs=1: written once, read later)
        qT_sb = singles.tile([128, HPC, QL], BF16)  # Q^T per head: [d, h, q]
        kT_sb = singles.tile([128, KV], BF16)  # K^T: [d, kv]
        v_sb = singles.tile([128, NKV, D], BF16)  # V: [kv%128, r, d]
        attnT_sb = singles.tile([128, HPC, QL], BF16)  # normalized attn^T per head

        def stream_dma(out, in_):
            nc.gpsimd.dma_start(out=out, in_=in_)

        def evac(i, out, in_):
            if i % 2 == 0:
                nc.scalar.copy(out=out, in_=in_)
            else:
                nc.vector.tensor_copy(out=out, in_=in_)

        def rms_scale(xg, nchunk, dlen, sqj, ssum):
            """xg: [128, nchunk, dlen] f32 view (norm over dlen cols of each
            chunk); scales each chunk slice by 1/rms in place."""
            sq2 = sqj[:, :, :].rearrange("p a b -> p (a b)")
            for c in range(nchunk):
                nc.scalar.activation(out=sq2[:, 0:dlen], in_=xg[:, c, 0:dlen],
                                     func=mybir.ActivationFunctionType.Square,
                                     accum_out=ssum[:, c:c + 1])
            # ssum := 1/rms = exp(-0.5*ln(mean + eps)); ln/exp/square live in
            # one ACT table set, so the whole kernel needs no table switches.
            nc.scalar.activation(out=ssum, in_=ssum, func=mybir.ActivationFunctionType.Ln,
                                 bias=epst, scale=1.0 / dlen)
            nc.scalar.activation(out=ssum, in_=ssum, func=mybir.ActivationFunctionType.Exp,
                                 bias=zbias, scale=-0.5)
            for c in range(nchunk):
                nc.vector.tensor_scalar_mul(out=xg[:, c, 0:dlen], in0=xg[:, c, 0:dlen],
                                            scalar1=ssum[:, c:c + 1])

        def rope4(xg, cs4):
            """xg: [128, 4, D] normalized input view; cs4: [128, 4, 2D] folded
            (cA|sA|cB|sB). In-place: 6 fused DVE ops."""
            x1 = xg[:, :, 0:64]
            x2 = xg[:, :, 64:128]
            t1 = tmp_pool.tile([128, 4, 64], F32, tag="t1")
            nc.vector.tensor_mul(t1, x2, cs4[:, :, 64:128])  # x2*sA
            t2 = tmp_pool.tile([128, 4, 64], F32, tag="t1")
            nc.vector.tensor_mul(t2, x1, cs4[:, :, 192:256])  # x1*sB
            nc.vector.tensor_mul(x1, x1, cs4[:, :, 0:64])  # x1*cA
            nc.vector.tensor_sub(x1, x1, t1)
            nc.vector.tensor_mul(x2, x2, cs4[:, :, 128:192])  # x2*cB
            nc.vector.tensor_add(x2, x2, t2)

        # ---- projections, Q and K/V groups interleaved ----
        def q_group(qg):
            pq = [acc_psum.tile([128, HPC * D], F32, tag="acc" if i < 2 else "po",
                    name=f"pq{qg}_{i}") for i in range(4)]
            for k4 in range(NHD // 4):
                cqt = ckq_pool.tile([128, 4, 512], BF16, tag="ckq")
                stream_dma(
                    cqt,
                    ckT[k4 * 512:(k4 + 1) * 512,
                        CTX + qg * 512: CTX + (qg + 1) * 512].rearrange(
                        "(four p) c -> p four c", p=128))
                for four in range(4):
                    k = 4 * k4 + four
                    for q4 in range(4):
                        nc.tensor.matmul(pq[q4], lhsT=cqt[:, four, q4 * 128:(q4 + 1) * 128],
                                         rhs=wq_sb[:, k, :], start=(k == 0), stop=(k == NHD - 1))
            for q4 in range(4):
                qc = qg * 4 + q4
                qe = evac_pool.tile([128, HPC * D], F32, tag="evac")
                evac(q4, qe, pq[q4])
                cst = cs_pool.tile([128, 2 * D], F32, tag="csq")
                nc.gpsimd.dma_start(out=cst, in_=csq[qc * 128:(qc + 1) * 128, :])
                sqj = tmp_pool.tile([128, 4, 64], F32, tag="t1")
                ssum = tmp_pool.tile([128, HPC], F32, tag="ssum")
                qe4 = qe.rearrange("p (h d) -> p h d", d=D)
                rms_scale(qe4, HPC, D, sqj, ssum)
                rope4(qe4, cst[:, :].rearrange("p (two d) -> p two d", two=1)
                      .broadcast_to([128, HPC, 2 * D]))
                for h in range(HPC):
                    tp = st_psum.tile([128, 128], F32, tag="st")
                    nc.tensor.transpose(tp, qe4[:, h, :], ident)
                    evac(h, qT_sb[:, h, qc * 128:(qc + 1) * 128], tp)

        def kv_group(rg):
            pk = [acc_psum.tile([128, 2 * D], F32, tag="acc" if i < 2 else "po",
                    name=f"pk{rg}_{i}") for i in range(4)]
            for k4 in range(NHD // 4):
                ckt = ckq_pool.tile([128, 4, 512], BF16, tag="ckq")
                stream_dma(
                    ckt,
                    ckT[k4 * 512:(k4 + 1) * 512,
                        rg * 512:(rg + 1) * 512].rearrange("(four p) c -> p four c", p=128))
                for four in range(4):
                    k = 4 * k4 + four
                    for r4 in range(4):
                        nc.tensor.matmul(pk[r4], lhsT=ckt[:, four, r4 * 128:(r4 + 1) * 128],
                                         rhs=wkv_sb[:, k, :], start=(k == 0), stop=(k == NHD - 1))
            ke4 = evac_pool.tile([128, 4, 2 * D], F32, tag="ke4")
            for r4 in range(4):
                evac(r4, ke4[:, r4, :], pk[r4])
            cst = cs_pool.tile([128, 4, 2 * D], F32, tag="csk")
            nc.gpsimd.dma_start(
                out=cst, in_=csk[rg * 512:(rg + 1) * 512, :].rearrange("(four p) d -> p four d", p=128))
            sqj = tmp_pool.tile([128, 4, 64], F32, tag="t1")
            ssum = tmp_pool.tile([128, 4], F32, tag="ssum")
            rms_scale(ke4[:, :, 0:D], 4, D, sqj, ssum)
            rope4(ke4[:, :, 0:D], cst)
            for r4 in range(4):
                r = rg * 4 + r4
                tp = st_psum.tile([128, 128], F32, tag="st")
                nc.tensor.transpose(tp, ke4[:, r4, 0:D], ident)
                evac(r4, kT_sb[:, r * 128:(r + 1) * 128], tp)
                nc.vector.tensor_copy(out=v_sb[:, r, :], in_=ke4[:, r4, D:2 * D])

        # interleave: [kv, kv, kv, q] keeps DMA wire and PE smoothly loaded;
        # the groups attention pass 0 does NOT need (kv chunks 40-47, q tiles
        # j in {2,3}) are emitted later at demoted priority, filling PE idle
        # during the ACT-bound pass 0.
        kv_group(0)
        nc.scalar.dma_start(out=wq_sb[:],
                            in_=wq[:, :].rearrange("(k p) n -> p k n", p=128))
        kv_group(1)
        kv_group(2)
        q_group(0)
        for r in (3, 4, 5):
            kv_group(r)
        q_group(1)
        for r in (6, 7, 8, 9):
            kv_group(r)

        # weights/masks needed by later phases: load while projections run
        wo_sb = singles.tile([128, HPC, HID], BF16)
        nc.sync.dma_start(out=wo_sb[:], in_=wo[:, :].rearrange("(h p) n -> p h n", p=128))
        msk_sb = singles.tile([128, 4, 512], BF16)
        nc.sync.dma_start(out=msk_sb[:], in_=msk[:, :].rearrange("p (i c) -> p i c", i=4))
        tri1 = singles.tile([128, 128], BF16)
        nc.sync.dma_start(out=tri1[:], in_=trio[:, :])

        # ---- attention ----
        # S^T orientation: [kv partitions, q free]; exp output IS P^T; PV with
        # V stationary gives out^T [d, q] directly.  One pass per j tile, two
        # heads fused per pass ([128,1024] ACTs, shared kT/v LDWEIGHTS); after
        # tile j finishes for all heads, its o_proj quarter (tag "po" PSUM, 2
        # banks) overlaps pass j+1's ACT-bound stretch.
        def attn_pass(g, h):
            j0 = 2 * g
            o_acc = {j: acc_psum.tile([128, 512], F32, tag="acc", name=f"oacc{h}_{j}")
                     for j in (j0, j0 + 1)}
            sacc = sacc_pool.tile([128, 1024], BF16, tag="sacc", name=f"sacc{h}_{g}")
            rmax = 39 + 8 * g
            for r in range(rmax + 1):
                js = [j for j in (j0, j0 + 1) if r <= 35 + 4 * j]
                st = st_psum.tile([128, 1024], F32, tag="st", name=f"st{h}_{r}_{g}")
                for j in js:
                    sl = (j - j0) * 512
                    i = r - 32 - 4 * j
                    diag = 0 <= i <= 3
                    nc.tensor.matmul(st[:, sl:sl + 512],
                                     lhsT=kT_sb[:, r * 128:(r + 1) * 128],
                                     rhs=qT_sb[:, h, j * 512:(j + 1) * 512],
                                     start=True, stop=not diag)
                    if diag:
                        nc.tensor.matmul(st[:, sl:sl + 512], lhsT=tri1,
                                         rhs=msk_sb[:, i, :], start=False, stop=True)

                lo = (js[0] - j0) * 512
                hi = (js[-1] - j0 + 1) * 512
                pt = p_pool.tile([128, 1024], BF16, tag="pt", name=f"pt{h}_{r}_{g}")
                nc.scalar.activation(out=pt[:, lo:hi], in_=st[:, lo:hi],
                                     func=mybir.ActivationFunctionType.Exp,
                                     bias=zbias, scale=SCALE)
                if r == 0:
                    nc.vector.tensor_copy(out=sacc[:, lo:hi], in_=pt[:, lo:hi])
                else:
                    nc.vector.tensor_add(sacc[:, lo:hi], sacc[:, lo:hi], pt[:, lo:hi])
                for j in js:
                    sl = (j - j0) * 512
                    nc.tensor.matmul(o_acc[j], lhsT=v_sb[:, r, :], rhs=pt[:, sl:sl + 512],
                                     start=(r == 0), stop=(r == 35 + 4 * j or r == NKV - 1))
                for j in js:
                    if r == (35 + 4 * j if j < NQJ - 1 else NKV - 1):
                        # softmax denominator, normalize into attnT (bf16)
                        sl = (j - j0) * 512
                        pr = stg_pool.tile([128, 512], F32, tag="pr", name=f"pr{h}_{j}")
                        nc.gpsimd.partition_all_reduce(pr, sacc[:, sl:sl + 512],
                                                       channels=128,
                                                       reduce_op=bass_isa.ReduceOp.add)
                        nc.vector.reciprocal(pr, pr)
                        nc.vector.tensor_mul(attnT_sb[:, h, j * 512:(j + 1) * 512],
                                             o_acc[j], pr)

        def oproj_qc(qc, tail=False):
            # out[qc] partial over this core's 512 attn features.  Overlapped
            # with attention only 2 PSUM banks (tag "po") are free; in the
            # tail the o_acc banks (tag "acc") are released too, so groups of
            # 4 amortize the evacuation stall, and the idle scalar engine
            # takes half the evacs + the stores.
            jq, qo = qc // 4, (qc % 4) * 128
            width = 4 if tail else 2
            for nsg in range(8 // width):
                po = [acc_psum.tile([128, 512], F32, tag=("po" if i < 2 else "acc"),
                                    name=f"po{qc}_{nsg}_{i}") for i in range(width)]
                ots = [ot_pool.tile([128, 1024], BF16, tag="ot", name=f"ot{qc}_{nsg}_{i}")
                       for i in range(width // 2)]
                for hh in range(HPC):
                    for nsw in range(width):
                        ns = nsg * width + nsw
                        nc.tensor.matmul(po[nsw],
                                         lhsT=attnT_sb[:, hh, jq * 512 + qo: jq * 512 + qo + 128],
                                         rhs=wo_sb[:, hh, ns * 512:(ns + 1) * 512],
                                         start=(hh == 0), stop=(hh == HPC - 1))
                for nsw in range(width):
                    dst = ots[nsw // 2][:, (nsw % 2) * 512:(nsw % 2 + 1) * 512]
                    if tail and nsw % 2 == 0:
                        nc.scalar.copy(out=dst, in_=po[nsw])
                    else:
                        nc.vector.tensor_copy(out=dst, in_=po[nsw])
                eng = nc.scalar if tail else nc.gpsimd
                for i, ot in enumerate(ots):
                    eng.dma_start(
                        out=out_ext[qc * 128:(qc + 1) * 128,
                                    (nsg * width + 2 * i) * 512:(nsg * width + 2 * i + 2) * 512],
                        in_=ot)

        with tc.high_priority(offset=-(10 ** 7)):
            q_group(2)
            q_group(3)
            kv_group(10)
            kv_group(11)
        for h in range(HPC):
            attn_pass(0, h)
        # pass 1 heads interleaved with o_proj of the finished j in {0,1} half
        for h in range(HPC):
            attn_pass(1, h)
            for qc in (2 * h, 2 * h + 1):
                oproj_qc(qc)
        for qc in range(8, NQC):
            oproj_qc(qc, tail=True)

    nc.compile()
    return nc


def _fold_cs(pos, nwv):
    """RoPE cos/sin tables with the per-head RMSNorm weight folded in:
    layout [len(pos), cA|sA|cB|sB] each 64 wide."""
    half = D // 2
    inv_freq = (1.0 / (THETA ** (np.arange(0, half, dtype=np.float32) / half))).astype(np.float32)
    fr = pos[:, None].astype(np.float32) * inv_freq[None, :]
    c, s = np.cos(fr), np.sin(fr)
    nw1, nw2 = nwv[None, :half], nwv[None, half:]
    return np.concatenate([c * nw1, s * nw2, c * nw2, s * nw1], axis=1).astype(np.float32)


def _host_prep(context, query, w_qkv, w_o, q_norm_w, k_norm_w):
    context = np.asarray(context, dtype=np.float32)
    query = np.asarray(query, dtype=np.float32)
    w_qkv = np.asarray(w_qkv, dtype=np.float32)
    w_o = np.asarray(w_o, dtype=np.float32)
    q_norm_w = np.asarray(q_norm_w, dtype=np.float32)
    k_norm_w = np.asarray(k_norm_w, dtype=np.float32)

    ck = np.concatenate([context, query], axis=0)  # [KV, HID]
    ckT = np.ascontiguousarray(ck.T).astype(bfloat16)  # [HID, KV]

    wq = w_qkv[:, :H * D]
    wk = w_qkv[:, H * D:H * D + KVH * D]
    wv = w_qkv[:, H * D + KVH * D:]

    csq = _fold_cs(np.arange(CTX, CTX + QL), q_norm_w)  # [QL, 2D]
    csk = _fold_cs(np.arange(KV), k_norm_w)  # [KV, 2D]

    k = np.arange(128)[:, None]
    q = np.arange(512)[None, :]
    # st[p,q] += sum_k tri[k,p] * msk_i[k,q] = MASKVAL * #{k <= p : q < k + 128i}
    # which is <= MASKVAL exactly on causally-masked elements, 0 elsewhere.
    msk = np.concatenate(
        [np.where(q < k + 128 * i, MASKVAL, 0.0) for i in range(4)],
        axis=1).astype(bfloat16)  # [128, 2048]
    trio = np.tril(np.ones((128, 128), np.float32)).T.astype(bfloat16)  # [k, p] : k <= p

    in_maps = []
    for c in range(NCORES):
        in_maps.append({
            "ckT": ckT,
            "wq": np.ascontiguousarray(wq[:, c * HPC * D:(c + 1) * HPC * D]).astype(bfloat16),
            "wkv": np.ascontiguousarray(
                np.concatenate([wk[:, c * D:(c + 1) * D], wv[:, c * D:(c + 1) * D]], axis=1)
            ).astype(bfloat16),
            "wo": np.ascontiguousarray(w_o[c * HPC * D:(c + 1) * HPC * D, :]).astype(bfloat16),
            "csq": csq,
            "csk": csk,
            "msk": msk,
            "trio": trio,
        })
    return in_maps


def kernel(context, query, w_qkv, w_o, q_norm_w, k_norm_w, **kw):
    if "nc" not in _STATE:
        _STATE["nc"] = _build()
    nc = _STATE["nc"]
    in_maps = _host_prep(context, query, w_qkv, w_o, q_norm_w, k_norm_w)
    res = run_bass_kernel_spmd(nc, in_maps, list(range(NCORES)), **kw)
    out = np.zeros((QL, HID), dtype=np.float32)
    for c in range(NCORES):
        out += np.asarray(res.results[c]["out"]).astype(np.float32)
    if kw:
        return out, res
    return out
